# revision 15
# baseline (speedup 1.0000x reference)
"""Multi-head self-attention (B=4, S=2048, D=768, H=12, dh=64) on 8 trn2 cores.

Sharding: core = b*2 + g  (b = batch 0..3, g = head-group of 6 heads).
Each core computes q/k/v projections for its 6 heads over the full sequence,
masked softmax attention, and a partial output projection (column slice of
o_w => row-parallel). Host sums the two partial outputs per batch element and
adds o_b once.

v3 design (vs the 220us baseline): PE work is cut ~30% via PE-array tiling,
with the attention unit split into phases so tiled matmuls never interleave
with open accumulation groups of a different tile mode (that pattern is
fatal on HW - NRT_EXEC_UNIT_UNRECOVERABLE):
  - S-phase: scores as K=64 row-tiled start/stop singles; the two heads of a
    pair (SBUF partitions 0..63 / 64..127) run in different PE row-groups
    CONCURRENTLY (~2x). kT is stored pair-stacked [128, pair, kv]; no
    zero-half padding, no kTz memset, single-op k eviction.
  - C-phase (pure, no hidden work inside): ctx as col-tiled open groups:
    sums matmul M=32 (v-lane col 0 = ones) -> psum rows 0..31, dims matmul
    M=64 (v-lane cols 64..127) -> psum rows 64..127; the two run in
    different PE col-groups concurrently (~2x). Score tiles stay [kv, q]
    so exp weights feed ctx as the moving operand with no transpose.
  - mask gather: only unmasked k positions (padded to a multiple of 128)
    are shipped/projected/exp'd; padding columns get a -1e30 per-partition
    bias inside the ACT exp (out = exp(scale*s + bias)).
  - exp: ACT for most kv chunks; DVE_KC chunks use a DVE Schraudolph
    tensor_scalar (bf16 bits in the low half of f32(k + 2^23), consumed as
    a stride-2 bitcast view) to keep ACT off the critical path.
  - out-projection chunks DMA psum -> DRAM directly (no DVE evict, no o_b
    broadcast; the host adds o_b once after summing the two partials).
  - hidden projection work (q/k/v proj, o proj) fills PE slack in S-phase
    slots and at phase boundaries only - every hidden chunk is a closed
    128-mode accumulation group, legal between tiled singles.
  - ONE psum pool for the whole program (tags "s" x2 + "c" x2 = 8 banks).
  - normalization tail as before: sums land in psum row 0 (readable by the
    custom-DVE reciprocal at partition offset 0), ctx dims in rows 64..127;
    the recip broadcast (K=128 ones-matmul) + normalize multiply are
    emitted in the NEXT unit's S-phase (128-mode, outside any C-phase).
"""

import numpy as np
import ml_dtypes

import concourse.bass as bass
import concourse.mybir as mybir
import concourse.tile as tile
from concourse import bacc
from concourse.bass_utils import run_bass_kernel_spmd

BS, SEQ, DIM, NH = 4, 2048, 768, 12
DH = 64
HEADS = 6            # heads per core
NPAIR = 3            # head pairs per core
DGRP = HEADS * DH    # 384
N_CORES = 8
P = 128
QH = 1024            # q-half width in the attention loop
KIN = DIM // P       # 6 contraction chunks for q/k proj
KIN_V = 7            # 768 inputs + ones row, padded to 896

F32 = mybir.dt.float32
BF16 = mybir.dt.bfloat16

MM_DT = BF16
MM_NP = ml_dtypes.bfloat16

NEG = -1.0e30
# ACT-path exp bias: centers the exact exp against the DVE piecewise-linear
# exp2 (max log2 ratio 0.08607 -> shift both means by half of that).
BIAS_CENTER = 0.0430365 * 0.6931471805599453   # = 0.0298296 (natural log)
# DVE Schraudolph constants: bf16 bits k = s * (128*log2e/8) + 16256,
# computed as  f32( s*23.08312 + (16256 + 2^23) )  -- the 2^23 add rounds
# k to the nearest integer in the f32 mantissa; bf16 bits are the LOW 16
# bits of the resulting f32 (element 0 of the bitcast pair).
SCH_SCALE = 128.0 * 1.4426950408889634 / 8.0   # 23.083120654
SCH_BIAS = 16256.0 + 8388608.0

# kv chunks whose exp runs on DVE instead of ACT (never the last, padded one)
DVE_KC = (5,)


def _build(NKV: int):
    """Build the per-core Bass program, parameterized by padded kv length."""
    KC = NKV // P          # kv chunks
    dve_kc = set(c for c in DVE_KC if c < KC - 1)

    nc = bacc.Bacc(None, target_bir_lowering=False, debug=False)

    xqT = nc.declare_dram_parameter("xqT", [DIM, SEQ], MM_DT, isOutput=False)
    xkT = nc.declare_dram_parameter("xkT", [DIM, NKV], MM_DT, isOutput=False)
    xvT = nc.declare_dram_parameter("xvT", [P * KIN_V, NKV], MM_DT, isOutput=False)
    wqT = nc.declare_dram_parameter("wqT", [DIM, DGRP], MM_DT, isOutput=False)
    wkT = nc.declare_dram_parameter("wkT", [DIM, DGRP], MM_DT, isOutput=False)
    wvT = nc.declare_dram_parameter("wvT", [P * KIN_V, DGRP], MM_DT, isOutput=False)
    woT = nc.declare_dram_parameter("woT", [DGRP, DIM], MM_DT, isOutput=False)
    qb = nc.declare_dram_parameter("qb", [DGRP], F32, isOutput=False)
    kb = nc.declare_dram_parameter("kb", [DGRP], F32, isOutput=False)
    pb = nc.declare_dram_parameter("pb", [NKV], F32, isOutput=False)
    out = nc.declare_dram_parameter("out", [SEQ, DIM], F32, isOutput=True)

    xqT_r = xqT.rearrange("(kk pi) n -> pi kk n", pi=P)
    xkT_r = xkT.rearrange("(kk pi) n -> pi kk n", pi=P)
    xvT_r = xvT.rearrange("(kk pi) n -> pi kk n", pi=P)
    wqT_r = wqT.rearrange("(kk pi) n -> pi kk n", pi=P)
    wkT_r = wkT.rearrange("(kk pi) n -> pi kk n", pi=P)
    wvT_r = wvT.rearrange("(kk pi) n -> pi kk n", pi=P)
    woT_r = woT.rearrange("(kk pi) n -> pi kk n", pi=P)
    qb_r = qb.rearrange("(m pi) -> pi m", pi=P)
    kb_r = kb.rearrange("(m pi) -> pi m", pi=P)
    pb_r = pb.rearrange("(c pi) -> pi c", pi=P)

    # k-proj slices along kv
    ksl = []
    o = 0
    while o < NKV:
        w = min(512, NKV - o)
        ksl.append((o, w))
        o += w

    with tile.TileContext(nc) as tc:
        with (
            tc.tile_pool(name="const", bufs=1) as const,
            tc.tile_pool(name="persist", bufs=1) as persist,
            tc.tile_pool(name="expp", bufs=4) as expp,
            tc.tile_pool(name="schp", bufs=4) as schp,
            tc.tile_pool(name="outp", bufs=5) as outp,
            tc.tile_pool(name="ps", bufs=2, space="PSUM") as ps,
        ):
            # ---- DMA order = first-use order; wq/xq0 split so the first
            # q-proj matmuls start as early as possible ----
            wq_sb = const.tile([P, KIN, DGRP], MM_DT)
            nc.sync.dma_start(wq_sb[:, 0:3, :], wqT_r[:, 0:3, :])
            xq_sb = persist.tile([P, KIN, SEQ], MM_DT)
            nc.sync.dma_start(xq_sb[:, 0:3, 0:512], xqT_r[:, 0:3, 0:512])
            nc.scalar.dma_start(wq_sb[:, 3:6, :], wqT_r[:, 3:6, :])
            nc.scalar.dma_start(xq_sb[:, 3:6, 0:512], xqT_r[:, 3:6, 0:512])
            qb_sb = const.tile([P, 3], F32)
            nc.sync.dma_start(qb_sb[:], qb_r)
            nc.scalar.dma_start(xq_sb[:, 0:3, 512:1024], xqT_r[:, 0:3, 512:1024])
            nc.sync.dma_start(xq_sb[:, 3:6, 512:1024], xqT_r[:, 3:6, 512:1024])
            wk_sb = const.tile([P, KIN, DGRP], MM_DT)
            nc.scalar.dma_start(wk_sb[:, 0:3, :], wkT_r[:, 0:3, :])
            nc.sync.dma_start(wk_sb[:, 3:6, :], wkT_r[:, 3:6, :])
            xk_sb = persist.tile([P, KIN, NKV], MM_DT)
            o0, w = ksl[0]
            nc.scalar.dma_start(xk_sb[:, 0:3, o0:o0 + w], xkT_r[:, 0:3, o0:o0 + w])
            nc.sync.dma_start(xk_sb[:, 3:6, o0:o0 + w], xkT_r[:, 3:6, o0:o0 + w])
            kb_sb = const.tile([P, 3], F32)
            nc.sync.dma_start(kb_sb[:], kb_r)
            pb_sb = const.tile([P, KC], F32)
            nc.sync.dma_start(pb_sb[:], pb_r)
            wv_sb = const.tile([P, KIN_V, DGRP], MM_DT)
            nc.sync.dma_start(wv_sb[:], wvT_r)
            xv_sb = persist.tile([P, KIN_V, NKV], MM_DT)
            nc.sync.dma_start(xv_sb[:, :, 0:256], xvT_r[:, :, 0:256])
            if len(ksl) > 1:
                o0, w = ksl[1]
                nc.sync.dma_start(xk_sb[:, :, o0:o0 + w], xkT_r[:, :, o0:o0 + w])
            nc.sync.dma_start(xv_sb[:, :, 256:640], xvT_r[:, :, 256:640])
            for o0, w in ksl[2:]:
                nc.sync.dma_start(xk_sb[:, :, o0:o0 + w], xkT_r[:, :, o0:o0 + w])
            nc.sync.dma_start(xv_sb[:, :, 640:NKV], xvT_r[:, :, 640:NKV])
            wo_sb = const.tile([P, 3, DIM], MM_DT)
            nc.sync.dma_start(wo_sb[:], woT_r)
            nc.sync.dma_start(xq_sb[:, :, 1024:1536], xqT_r[:, :, 1024:1536])
            nc.sync.dma_start(xq_sb[:, :, 1536:2048], xqT_r[:, :, 1536:2048])

            # dummy exp: pulls the ~2.7us ACT_TABLE_LOAD into the DMA ramp
            warm_t = const.tile([P, 1], F32)
            nc.scalar.activation(warm_t[:], qb_sb[:, 0:1],
                                 mybir.ActivationFunctionType.Exp)

            # ---- persistent activations ----
            qT_sb = persist.tile([P, NPAIR, SEQ], MM_DT)
            # kT pair-stacked: rows 0..63 = head 2m, rows 64..127 = head 2m+1
            kT_sb = persist.tile([P, NPAIR, NKV], MM_DT)
            # v lanes are 128 wide per head: col 0 = ones (sums matmul lhsT is
            # cols 0..31 -> softmax sums land in psum row 0, readable by the
            # custom-DVE recip), cols 64..127 = the 64 v dims (dims matmul
            # lhsT -> psum rows 64..127). cols 1..63 never read.
            v_sb = persist.tile([P, KC, HEADS * 128], MM_DT)
            ctx_sb = persist.tile([P, NPAIR, SEQ], MM_DT)
            # recip tiles (row 0 = 1/sums, rows 1..127 = 0 so the K=128
            # ones-matmul broadcast never multiplies 0 by inf/nan garbage)
            recipA = persist.tile([P, QH], F32)
            recipB = persist.tile([P, QH], F32)
            # partition-broadcast targets (Pool writes, DVE mult reads)
            rbA_sb = persist.tile([P, QH], F32)
            rbB_sb = persist.tile([P, QH], F32)
            for h in range(HEADS):
                nc.vector.memset(v_sb[:, :, 128 * h], 1.0)

            # ---- emission helpers (all share the "s"/"c" psum slots) ----
            def q_chunk(m, nt, tag="c"):
                ps_t = ps.tile([P, QH], F32, tag=tag, name=f"q{m}{nt}")
                for kk in range(KIN):
                    nc.tensor.matmul(
                        ps_t[:, 0:512],
                        wq_sb[:, kk, m * P:(m + 1) * P],
                        xq_sb[:, kk, nt * 512:(nt + 1) * 512],
                        start=(kk == 0), stop=(kk == KIN - 1),
                    )
                nc.vector.tensor_scalar_add(
                    qT_sb[:, m, nt * 512:(nt + 1) * 512], ps_t[:, 0:512],
                    qb_sb[:, m, None],
                )

            def k_chunk(m, isl, tag="c"):
                o0, w = ksl[isl]
                ps_t = ps.tile([P, QH], F32, tag=tag, name=f"k{m}{isl}")
                for kk in range(KIN):
                    nc.tensor.matmul(
                        ps_t[:, 0:w],
                        wk_sb[:, kk, m * P:(m + 1) * P],
                        xk_sb[:, kk, o0:o0 + w],
                        start=(kk == 0), stop=(kk == KIN - 1),
                    )
                nc.vector.tensor_scalar_add(
                    kT_sb[:, m, o0:o0 + w], ps_t[:, 0:w],
                    kb_sb[:, m, None],
                )

            def v_chunk(c, tag="c"):
                ps_t = ps.tile([P, QH], F32, tag=tag, name=f"v{c}")
                for kk in range(KIN_V):
                    nc.tensor.matmul(
                        ps_t[:, 0:DGRP],
                        xv_sb[:, kk, c * P:(c + 1) * P],
                        wv_sb[:, kk, :],
                        start=(kk == 0), stop=(kk == KIN_V - 1),
                    )
                # strided evict into slots 64..127 of the 128-wide head lanes
                nc.vector.tensor_copy(
                    out=v_sb[:, c, :].rearrange("p (h x) -> p h x", x=128)[:, :, 64:128],
                    in_=ps_t[:, 0:DGRP].rearrange("p (h x) -> p h x", x=64),
                )

            def o_chunk(qc, tag="c", dma=None):
                ps_t = ps.tile([P, QH], F32, tag=tag, name=f"o{qc}")
                for kk in range(3):
                    for n0, nsz in ((0, 512), (512, 256)):
                        nc.tensor.matmul(
                            ps_t[:, n0:n0 + nsz],
                            ctx_sb[:, kk, qc * P:(qc + 1) * P],
                            wo_sb[:, kk, n0:n0 + nsz],
                            start=(kk == 0), stop=(kk == 2),
                        )
                # DMA cannot read PSUM: stage through SBUF with a plain copy
                # (host adds o_b once after summing the two partials)
                o_t = outp.tile([P, DIM], F32, tag="o", name=f"ot{qc}")
                nc.vector.tensor_copy(out=o_t[:], in_=ps_t[:, 0:DIM])
                (dma or nc.sync).dma_start(out[qc * P:(qc + 1) * P, :], o_t[:])

            def unit_tail(pair, qh, cE, cO):
                # normalization tail: approx recip off psum row 0 (custom-DVE
                # ops require partition-0 inputs), partition-broadcast of the
                # recip row on the IDLE Pool engine (no PE ones-matmul), ctx
                # dims staged to SBUF (TensorTensor allows only one PSUM
                # input), one multiply per head into ctx_sb.
                q0 = qh * QH
                nc.vector.reciprocal_approx_fast(
                    out=recipA[0:1, :], in_=cE[0:1, :])
                nc.gpsimd.partition_broadcast(rbA_sb[:], recipA[0:1, :])
                # one staging tile, E dims in rows 0..63 / O dims in 64..127,
                # so each SBUF-only multiply has aligned partition ranges
                cu2 = expp.tile([P, QH], MM_DT, tag="ctxu", bufs=2,
                                name=f"cu{qh}{pair}")
                nc.vector.tensor_copy(out=cu2[0:DH, :], in_=cE[64:128, :])
                nc.vector.reciprocal_approx_fast(
                    out=recipB[0:1, :], in_=cO[0:1, :])
                nc.gpsimd.partition_broadcast(rbB_sb[:], recipB[0:1, :])
                nc.vector.tensor_copy(out=cu2[DH:P, :], in_=cO[64:128, :])
                nc.vector.tensor_tensor(
                    ctx_sb[0:DH, pair, q0:q0 + QH],
                    cu2[0:DH, :], rbA_sb[0:DH, :], mybir.AluOpType.mult,
                )
                nc.vector.tensor_tensor(
                    ctx_sb[DH:P, pair, q0:q0 + QH],
                    cu2[DH:P, :], rbB_sb[DH:P, :], mybir.AluOpType.mult,
                )

            units = [(qh, pr) for qh in range(2) for pr in range(NPAIR)]
            hidden: dict = {
                # order matters: v chunk c must be emitted before the C-phase
                # that reads it (same unit); k/q chunks for a unit must be
                # emitted in an EARLIER unit (in-order PE queue: a score
                # matmul cannot wait on projection work emitted behind it).
                (0, 0): [lambda t: v_chunk(1, t), lambda t: v_chunk(2, t),
                         lambda t: k_chunk(0, 1, t), lambda t: v_chunk(3, t),
                         (lambda t: k_chunk(0, 2, t)) if len(ksl) > 2
                         else (lambda t: v_chunk(4, t))]
                        + [lambda t, c=c: v_chunk(c, t)
                           for c in range(4 if len(ksl) > 2 else 5, KC)]
                        + [lambda t: q_chunk(1, 0, t), lambda t: q_chunk(1, 1, t)]
                        + [lambda t, i=i: k_chunk(1, i, t)
                           for i in range(len(ksl))],
                (0, 1): [lambda t: q_chunk(2, 0, t), lambda t: q_chunk(2, 1, t)]
                        + [lambda t, i=i: k_chunk(2, i, t)
                           for i in range(len(ksl))]
                        + [lambda t: q_chunk(0, 2, t), lambda t: q_chunk(0, 3, t)],
                (0, 2): [lambda t: q_chunk(1, 2, t), lambda t: q_chunk(1, 3, t),
                         lambda t: q_chunk(2, 2, t), lambda t: q_chunk(2, 3, t)],
                (1, 0): [lambda t, qc=qc: o_chunk(qc, t) for qc in range(0, 5)],
                (1, 1): [lambda t, qc=qc: o_chunk(qc, t) for qc in range(5, 8)],
                (1, 2): [],
            }

            # lead-in: just enough projection to unblock unit (0, 0)
            q_chunk(0, 0, tag="c")
            q_chunk(0, 1, tag="c")
            k_chunk(0, 0, tag="c")
            # only v0..v8 are needed before the C-phase; v0 borrows a ctx psum
            # slot so the first score tiles don't queue behind its eviction.
            v_chunk(0, tag="c")

            for qh, pair in units:
                q0 = qh * QH
                work = list(hidden[(qh, pair)])
                weights = [{}, {}]      # per-parity exp views for ctx

                # ---- S phase: row-tiled score singles + exp + hidden ----
                for kc in range(KC):
                    # ONE psum tile for both parities: a single alloc-wait
                    # covers all 4 score matmuls, so the row-tiled E/O pairs
                    # issue back-to-back and run in different PE row-groups
                    # concurrently (separate tiles skewed E vs O by a full
                    # ACT exp epoch, killing the concurrency).
                    st2 = ps.tile([P, 2 * QH], F32, tag="s", bufs=1,
                                  name=f"st{qh}{pair}{kc}")
                    stE = st2[:, 0:QH]
                    stO = st2[:, QH:2 * QH]
                    for qt in range(2):
                        sl = slice(qt * 512, (qt + 1) * 512)
                        qsl = slice(q0 + qt * 512, q0 + (qt + 1) * 512)
                        nc.tensor.matmul(
                            stE[:, sl], kT_sb[0:64, pair, kc * P:(kc + 1) * P],
                            qT_sb[0:64, pair, qsl], start=True, stop=True)
                        nc.tensor.matmul(
                            stO[:, sl], kT_sb[64:128, pair, kc * P:(kc + 1) * P],
                            qT_sb[64:128, pair, qsl], start=True, stop=True)
                    # exp: ACT or DVE (Schraudolph) by kv chunk
                    for par, st in ((0, stE), (1, stO)):
                        st = st[:, :]
                        if kc in dve_kc:
                            sch_t = schp.tile([P, QH], F32, tag="sch",
                                              bufs=5,
                                              name=f"sch{qh}{pair}{kc}{par}")
                            nc.vector.tensor_scalar(
                                sch_t[:], st[:],
                                SCH_SCALE, SCH_BIAS,
                                mybir.AluOpType.mult, mybir.AluOpType.add,
                            )
                            weights[par][kc] = sch_t.bitcast(MM_DT).rearrange(
                                "p (n two) -> p n two", two=2)[:, :, 0]
                        else:
                            # all of a unit's exp tiles stay live through its
                            # C-phase (in-order PE queue: an alloc that waits
                            # on a C-phase reader behind it would deadlock)
                            exp_t = expp.tile([P, QH], MM_DT, tag="exp",
                                              bufs=17,
                                              name=f"exp{qh}{pair}{kc}{par}")
                            nc.scalar.activation(
                                exp_t[:], st[:],
                                mybir.ActivationFunctionType.Exp,
                                bias=pb_sb[:, kc, None], scale=0.125,
                            )
                            weights[par][kc] = exp_t
                    # hidden projection / out-projection chunks; pop two
                    # per slot when the list would otherwise not fit. Pops
                    # start at kc==2 only: an o_chunk popped earlier would
                    # read ctx_sb rows the kc==2 pending_fin hasn't written.
                    if kc >= 2 and work:
                        work.pop(0)("c")
                        if len(work) > KC - 1 - kc:
                            work.pop(0)("c")
                while work:
                    work.pop(0)("c")

                # ---- C phase: col-tiled ctx accumulation (PURE: no other
                # matmuls may be emitted until the groups close) ----
                cE = ps.tile([P, QH], F32, tag="c", name=f"cE{qh}{pair}")
                cO = ps.tile([P, QH], F32, tag="c", name=f"cO{qh}{pair}")
                # M=128 single matmuls (col-tiled sums/dims pairs measured
                # SLOWER: same-row LDWEIGHTS can't overlap the in-flight
                # matmul, serializing at ~330ns vs 216ns streamed)
                for par, cX in ((0, cE), (1, cO)):
                    h = 2 * pair + par
                    for ckc in range(KC):
                        wt = weights[par][ckc]
                        for qt in range(2):
                            sl = slice(qt * 512, (qt + 1) * 512)
                            nc.tensor.matmul(
                                cX[:, sl],
                                v_sb[:, ckc, 128 * h:128 * h + 128],
                                wt[:, sl],
                                start=(ckc == 0), stop=(ckc == KC - 1))

                unit_tail(pair, qh, cE, cO)

            # out-projection for the second q half (ACT's hwdge queue is
            # idle after the last exp; split the issue across both queues)
            for qc in range(8, 16):
                o_chunk(qc, dma=nc.scalar if qc % 2 else nc.sync)

    nc.compile()
    return nc


_cache: dict = {}

# test harnesses may set e.g. {"trace": True, "tmpdir": ...}; empty for grading
_run_opts: dict = {}
LAST_RES = None


def _get_nc(NKV: int):
    if NKV not in _cache:
        _cache[NKV] = _build(NKV)
    return _cache[NKV]


def kernel(query, key_, value, mask, q_w, q_b, k_w, k_b, v_w, v_b, o_w, o_b):
    query = np.asarray(query, np.float32)
    key_ = np.asarray(key_, np.float32)
    value = np.asarray(value, np.float32)
    mask = np.asarray(mask)
    q_w = np.asarray(q_w, np.float32)
    q_b = np.asarray(q_b, np.float32)
    k_w = np.asarray(k_w, np.float32)
    k_b = np.asarray(k_b, np.float32)
    v_w = np.asarray(v_w, np.float32)
    v_b = np.asarray(v_b, np.float32)
    o_w = np.asarray(o_w, np.float32)
    o_b = np.asarray(o_b, np.float32)

    counts = (mask != 0).sum(axis=1)
    NKV = max(P, int(-(-int(counts.max()) // P) * P))
    nc = _get_nc(NKV)

    in_maps = []
    for b in range(BS):
        idx = np.nonzero(mask[b])[0]
        cnt = len(idx)
        xk_g = np.zeros((NKV, DIM), np.float32)
        xv_g = np.zeros((NKV, DIM), np.float32)
        xk_g[:cnt] = key_[b][idx]
        xv_g[:cnt] = value[b][idx]
        xqT_b = np.ascontiguousarray(query[b].T).astype(MM_NP)
        xkT_b = np.ascontiguousarray(xk_g.T).astype(MM_NP)
        xvT_b = np.zeros((P * KIN_V, NKV), MM_NP)
        xvT_b[:DIM] = xv_g.T
        xvT_b[DIM] = 1.0
        pb_b = np.where(np.arange(NKV) < cnt, BIAS_CENTER, NEG).astype(np.float32)
        for g in range(2):
            sl = slice(DGRP * g, DGRP * (g + 1))
            in_maps.append({
                "xqT": xqT_b,
                "xkT": xkT_b,
                "xvT": xvT_b,
                "wqT": np.ascontiguousarray(q_w[sl].T).astype(MM_NP),
                "wkT": np.ascontiguousarray(k_w[sl].T).astype(MM_NP),
                "wvT": np.concatenate(
                    [v_w[sl].T, v_b[None, sl],
                     np.zeros((P - 1, DGRP), np.float32)], axis=0).astype(MM_NP),
                "woT": np.ascontiguousarray(o_w[:, sl].T).astype(MM_NP),
                "qb": q_b[sl].copy(),
                "kb": k_b[sl].copy(),
                "pb": pb_b,
            })

    res = run_bass_kernel_spmd(nc, in_maps, core_ids=list(range(N_CORES)),
                               **_run_opts)
    global LAST_RES
    LAST_RES = res
    out = np.empty((BS, SEQ, DIM), np.float32)
    for b in range(BS):
        out[b] = res.results[2 * b]["out"] + res.results[2 * b + 1]["out"] + o_b
    return out


# revision 16
# speedup vs baseline: 1.1554x; 1.1554x over previous
"""Multi-head self-attention (B=4, S=2048, D=768, H=12, dh=64) on 8 trn2 cores.

Sharding: core = b*2 + g  (b = batch 0..3, g = head-group of 6 heads).
Each core computes q/k/v projections for its 6 heads over the full sequence,
masked softmax attention, and a partial output projection (column slice of
o_w => row-parallel). Host sums the two partial outputs per batch element and
adds o_b once.

v3 design (vs the 220us baseline): PE work is cut ~30% via PE-array tiling,
with the attention unit split into phases so tiled matmuls never interleave
with open accumulation groups of a different tile mode (that pattern is
fatal on HW - NRT_EXEC_UNIT_UNRECOVERABLE):
  - S-phase: scores as K=64 row-tiled start/stop singles; the two heads of a
    pair (SBUF partitions 0..63 / 64..127) run in different PE row-groups
    CONCURRENTLY (~2x). kT is stored pair-stacked [128, pair, kv]; no
    zero-half padding, no kTz memset, single-op k eviction.
  - C-phase (pure, no hidden work inside): ctx as col-tiled open groups:
    sums matmul M=32 (v-lane col 0 = ones) -> psum rows 0..31, dims matmul
    M=64 (v-lane cols 64..127) -> psum rows 64..127; the two run in
    different PE col-groups concurrently (~2x). Score tiles stay [kv, q]
    so exp weights feed ctx as the moving operand with no transpose.
  - mask gather: only unmasked k positions (padded to a multiple of 128)
    are shipped/projected/exp'd; padding columns get a -1e30 per-partition
    bias inside the ACT exp (out = exp(scale*s + bias)).
  - exp: ACT for most kv chunks; DVE_KC chunks use a DVE Schraudolph
    tensor_scalar (bf16 bits in the low half of f32(k + 2^23), consumed as
    a stride-2 bitcast view) to keep ACT off the critical path.
  - out-projection chunks DMA psum -> DRAM directly (no DVE evict, no o_b
    broadcast; the host adds o_b once after summing the two partials).
  - hidden projection work (q/k/v proj, o proj) fills PE slack in S-phase
    slots and at phase boundaries only - every hidden chunk is a closed
    128-mode accumulation group, legal between tiled singles.
  - ONE psum pool for the whole program (tags "s" x2 + "c" x2 = 8 banks).
  - normalization tail as before: sums land in psum row 0 (readable by the
    custom-DVE reciprocal at partition offset 0), ctx dims in rows 64..127;
    the recip broadcast (K=128 ones-matmul) + normalize multiply are
    emitted in the NEXT unit's S-phase (128-mode, outside any C-phase).
"""

import numpy as np
import ml_dtypes

import concourse.bass as bass
import concourse.mybir as mybir
import concourse.tile as tile
from concourse import bacc
from concourse.bass_utils import run_bass_kernel_spmd

BS, SEQ, DIM, NH = 4, 2048, 768, 12
DH = 64
HEADS = 6            # heads per core
NPAIR = 3            # head pairs per core
DGRP = HEADS * DH    # 384
N_CORES = 8
P = 128
QH = 1024            # q-half width in the attention loop
KIN = DIM // P       # 6 contraction chunks for q/k proj
KIN_V = 7            # 768 inputs + ones row, padded to 896

F32 = mybir.dt.float32
BF16 = mybir.dt.bfloat16

MM_DT = BF16
MM_NP = ml_dtypes.bfloat16

NEG = -1.0e30
# ACT-path exp bias: centers the exact exp against the DVE piecewise-linear
# exp2 (max log2 ratio 0.08607 -> shift both means by half of that).
BIAS_CENTER = 0.0430365 * 0.6931471805599453   # = 0.0298296 (natural log)
# DVE Schraudolph constants: bf16 bits k = s * (128*log2e/8) + 16256,
# computed as  f32( s*23.08312 + (16256 + 2^23) )  -- the 2^23 add rounds
# k to the nearest integer in the f32 mantissa; bf16 bits are the LOW 16
# bits of the resulting f32 (element 0 of the bitcast pair).
SCH_SCALE = 128.0 * 1.4426950408889634 / 8.0   # 23.083120654
SCH_BIAS = 16256.0 + 8388608.0

# kv chunks whose exp runs on DVE instead of ACT (never the last, padded one)
DVE_KC = (5,)


def _build(NKV: int):
    """Build the per-core Bass program, parameterized by padded kv length."""
    KC = NKV // P          # kv chunks
    dve_kc = set(c for c in DVE_KC if c < KC - 1)

    nc = bacc.Bacc(None, target_bir_lowering=False, debug=False)

    xqT = nc.declare_dram_parameter("xqT", [DIM, SEQ], MM_DT, isOutput=False)
    xkT = nc.declare_dram_parameter("xkT", [DIM, NKV], MM_DT, isOutput=False)
    xvT = nc.declare_dram_parameter("xvT", [P * KIN_V, NKV], MM_DT, isOutput=False)
    wqT = nc.declare_dram_parameter("wqT", [DIM, DGRP], MM_DT, isOutput=False)
    wkT = nc.declare_dram_parameter("wkT", [DIM, DGRP], MM_DT, isOutput=False)
    wvT = nc.declare_dram_parameter("wvT", [P * KIN_V, DGRP], MM_DT, isOutput=False)
    woT = nc.declare_dram_parameter("woT", [DGRP, DIM], MM_DT, isOutput=False)
    qb = nc.declare_dram_parameter("qb", [DGRP], F32, isOutput=False)
    kb = nc.declare_dram_parameter("kb", [DGRP], F32, isOutput=False)
    pb = nc.declare_dram_parameter("pb", [NKV], F32, isOutput=False)
    out = nc.declare_dram_parameter("out", [SEQ, DIM], F32, isOutput=True)

    xqT_r = xqT.rearrange("(kk pi) n -> pi kk n", pi=P)
    xkT_r = xkT.rearrange("(kk pi) n -> pi kk n", pi=P)
    xvT_r = xvT.rearrange("(kk pi) n -> pi kk n", pi=P)
    wqT_r = wqT.rearrange("(kk pi) n -> pi kk n", pi=P)
    wkT_r = wkT.rearrange("(kk pi) n -> pi kk n", pi=P)
    wvT_r = wvT.rearrange("(kk pi) n -> pi kk n", pi=P)
    woT_r = woT.rearrange("(kk pi) n -> pi kk n", pi=P)
    qb_r = qb.rearrange("(m pi) -> pi m", pi=P)
    kb_r = kb.rearrange("(m pi) -> pi m", pi=P)
    pb_r = pb.rearrange("(c pi) -> pi c", pi=P)

    # k-proj slices along kv
    ksl = []
    o = 0
    while o < NKV:
        w = min(512, NKV - o)
        ksl.append((o, w))
        o += w

    with tile.TileContext(nc) as tc:
        with (
            tc.tile_pool(name="const", bufs=1) as const,
            tc.tile_pool(name="persist", bufs=1) as persist,
            tc.tile_pool(name="expp", bufs=4) as expp,
            tc.tile_pool(name="schp", bufs=4) as schp,
            tc.tile_pool(name="outp", bufs=5) as outp,
            tc.tile_pool(name="ps", bufs=2, space="PSUM") as ps,
        ):
            # ---- DMA order = first-use order; wq/xq0 split so the first
            # q-proj matmuls start as early as possible ----
            wq_sb = const.tile([P, KIN, DGRP], MM_DT)
            nc.sync.dma_start(wq_sb[:, 0:3, :], wqT_r[:, 0:3, :])
            xq_sb = persist.tile([P, KIN, SEQ], MM_DT)
            nc.sync.dma_start(xq_sb[:, 0:3, 0:512], xqT_r[:, 0:3, 0:512])
            nc.scalar.dma_start(wq_sb[:, 3:6, :], wqT_r[:, 3:6, :])
            nc.scalar.dma_start(xq_sb[:, 3:6, 0:512], xqT_r[:, 3:6, 0:512])
            qb_sb = const.tile([P, 3], F32)
            nc.sync.dma_start(qb_sb[:], qb_r)
            nc.scalar.dma_start(xq_sb[:, 0:3, 512:1024], xqT_r[:, 0:3, 512:1024])
            nc.sync.dma_start(xq_sb[:, 3:6, 512:1024], xqT_r[:, 3:6, 512:1024])
            wk_sb = const.tile([P, KIN, DGRP], MM_DT)
            nc.scalar.dma_start(wk_sb[:, 0:3, :], wkT_r[:, 0:3, :])
            nc.sync.dma_start(wk_sb[:, 3:6, :], wkT_r[:, 3:6, :])
            xk_sb = persist.tile([P, KIN, NKV], MM_DT)
            o0, w = ksl[0]
            nc.scalar.dma_start(xk_sb[:, 0:3, o0:o0 + w], xkT_r[:, 0:3, o0:o0 + w])
            nc.sync.dma_start(xk_sb[:, 3:6, o0:o0 + w], xkT_r[:, 3:6, o0:o0 + w])
            kb_sb = const.tile([P, 3], F32)
            nc.sync.dma_start(kb_sb[:], kb_r)
            pb_sb = const.tile([P, KC], F32)
            nc.sync.dma_start(pb_sb[:], pb_r)
            wv_sb = const.tile([P, KIN_V, DGRP], MM_DT)
            nc.sync.dma_start(wv_sb[:], wvT_r)
            xv_sb = persist.tile([P, KIN_V, NKV], MM_DT)
            nc.sync.dma_start(xv_sb[:, :, 0:256], xvT_r[:, :, 0:256])
            if len(ksl) > 1:
                o0, w = ksl[1]
                nc.sync.dma_start(xk_sb[:, :, o0:o0 + w], xkT_r[:, :, o0:o0 + w])
            nc.sync.dma_start(xv_sb[:, :, 256:640], xvT_r[:, :, 256:640])
            for o0, w in ksl[2:]:
                nc.sync.dma_start(xk_sb[:, :, o0:o0 + w], xkT_r[:, :, o0:o0 + w])
            nc.sync.dma_start(xv_sb[:, :, 640:NKV], xvT_r[:, :, 640:NKV])
            wo_sb = const.tile([P, 3, DIM], MM_DT)
            nc.sync.dma_start(wo_sb[:], woT_r)
            nc.sync.dma_start(xq_sb[:, :, 1024:1536], xqT_r[:, :, 1024:1536])
            nc.sync.dma_start(xq_sb[:, :, 1536:2048], xqT_r[:, :, 1536:2048])

            # dummy exp: pulls the ~2.7us ACT_TABLE_LOAD into the DMA ramp
            warm_t = const.tile([P, 1], F32)
            nc.scalar.activation(warm_t[:], qb_sb[:, 0:1],
                                 mybir.ActivationFunctionType.Exp)

            # ---- persistent activations ----
            qT_sb = persist.tile([P, NPAIR, SEQ], MM_DT)
            # kT pair-stacked: rows 0..63 = head 2m, rows 64..127 = head 2m+1
            kT_sb = persist.tile([P, NPAIR, NKV], MM_DT)
            # v lanes are 128 wide per head: col 0 = ones (sums matmul lhsT is
            # cols 0..31 -> softmax sums land in psum row 0, readable by the
            # custom-DVE recip), cols 64..127 = the 64 v dims (dims matmul
            # lhsT -> psum rows 64..127). cols 1..63 never read.
            v_sb = persist.tile([P, KC, HEADS * 128], MM_DT)
            ctx_sb = persist.tile([P, NPAIR, SEQ], MM_DT)
            # recip tiles (row 0 = 1/sums, rows 1..127 = 0 so the K=128
            # ones-matmul broadcast never multiplies 0 by inf/nan garbage)
            recipA = persist.tile([P, QH], F32)
            recipB = persist.tile([P, QH], F32)
            # partition-broadcast targets (Pool writes, DVE mult reads)
            rbA_sb = persist.tile([P, QH], F32)
            rbB_sb = persist.tile([P, QH], F32)
            for h in range(HEADS):
                nc.vector.memset(v_sb[:, :, 128 * h], 1.0)

            # ---- emission helpers (all share the "s"/"c" psum slots) ----
            def q_chunk(m, nt, tag="c"):
                ps_t = ps.tile([P, QH], F32, tag=tag, name=f"q{m}{nt}")
                for kk in range(KIN):
                    nc.tensor.matmul(
                        ps_t[:, 0:512],
                        wq_sb[:, kk, m * P:(m + 1) * P],
                        xq_sb[:, kk, nt * 512:(nt + 1) * 512],
                        start=(kk == 0), stop=(kk == KIN - 1),
                    )
                nc.vector.tensor_scalar_add(
                    qT_sb[:, m, nt * 512:(nt + 1) * 512], ps_t[:, 0:512],
                    qb_sb[:, m, None],
                )

            def k_chunk(m, isl, tag="c"):
                o0, w = ksl[isl]
                ps_t = ps.tile([P, QH], F32, tag=tag, name=f"k{m}{isl}")
                for kk in range(KIN):
                    nc.tensor.matmul(
                        ps_t[:, 0:w],
                        wk_sb[:, kk, m * P:(m + 1) * P],
                        xk_sb[:, kk, o0:o0 + w],
                        start=(kk == 0), stop=(kk == KIN - 1),
                    )
                nc.vector.tensor_scalar_add(
                    kT_sb[:, m, o0:o0 + w], ps_t[:, 0:w],
                    kb_sb[:, m, None],
                )

            def v_chunk(c, tag="c"):
                ps_t = ps.tile([P, QH], F32, tag=tag, name=f"v{c}")
                for kk in range(KIN_V):
                    nc.tensor.matmul(
                        ps_t[:, 0:DGRP],
                        xv_sb[:, kk, c * P:(c + 1) * P],
                        wv_sb[:, kk, :],
                        start=(kk == 0), stop=(kk == KIN_V - 1),
                    )
                # strided evict into slots 64..127 of the 128-wide head lanes
                nc.vector.tensor_copy(
                    out=v_sb[:, c, :].rearrange("p (h x) -> p h x", x=128)[:, :, 64:128],
                    in_=ps_t[:, 0:DGRP].rearrange("p (h x) -> p h x", x=64),
                )

            def o_chunk(qc, tag="c", dma=None):
                ps_t = ps.tile([P, QH], F32, tag=tag, name=f"o{qc}")
                for kk in range(3):
                    for n0, nsz in ((0, 512), (512, 256)):
                        nc.tensor.matmul(
                            ps_t[:, n0:n0 + nsz],
                            ctx_sb[:, kk, qc * P:(qc + 1) * P],
                            wo_sb[:, kk, n0:n0 + nsz],
                            start=(kk == 0), stop=(kk == 2),
                        )
                # DMA cannot read PSUM: stage through SBUF with a plain copy
                # (host adds o_b once after summing the two partials)
                o_t = outp.tile([P, DIM], F32, tag="o", name=f"ot{qc}")
                nc.vector.tensor_copy(out=o_t[:], in_=ps_t[:, 0:DIM])
                (dma or nc.sync).dma_start(out[qc * P:(qc + 1) * P, :], o_t[:])

            def unit_tail(pair, qh, cE, cO):
                # normalization tail: approx recip off psum row 0 (custom-DVE
                # ops require partition-0 inputs), partition-broadcast of the
                # recip row on the IDLE Pool engine (no PE ones-matmul), ctx
                # dims staged to SBUF (TensorTensor allows only one PSUM
                # input), one multiply per head into ctx_sb.
                q0 = qh * QH
                nc.vector.reciprocal_approx_fast(
                    out=recipA[0:1, :], in_=cE[0:1, :])
                nc.gpsimd.partition_broadcast(rbA_sb[:], recipA[0:1, :])
                nc.vector.reciprocal_approx_fast(
                    out=recipB[0:1, :], in_=cO[0:1, :])
                nc.gpsimd.partition_broadcast(rbB_sb[:], recipB[0:1, :])
                nc.vector.tensor_tensor(
                    ctx_sb[0:DH, pair, q0:q0 + QH],
                    cE[64:128, :], rbA_sb[0:DH, :], mybir.AluOpType.mult,
                )
                nc.vector.tensor_tensor(
                    ctx_sb[DH:P, pair, q0:q0 + QH],
                    cO[64:128, :], rbB_sb[DH:P, :], mybir.AluOpType.mult,
                )

            units = [(qh, pr) for qh in range(2) for pr in range(NPAIR)]
            hidden: dict = {
                # order matters: v chunk c must be emitted before the C-phase
                # that reads it (same unit); k/q chunks for a unit must be
                # emitted in an EARLIER unit (in-order PE queue: a score
                # matmul cannot wait on projection work emitted behind it).
                (0, 0): [lambda t: v_chunk(1, t), lambda t: v_chunk(2, t),
                         lambda t: k_chunk(0, 1, t), lambda t: v_chunk(3, t),
                         (lambda t: k_chunk(0, 2, t)) if len(ksl) > 2
                         else (lambda t: v_chunk(4, t))]
                        + [lambda t, c=c: v_chunk(c, t)
                           for c in range(4 if len(ksl) > 2 else 5, KC)]
                        + [lambda t: q_chunk(1, 0, t), lambda t: q_chunk(1, 1, t)]
                        + [lambda t, i=i: k_chunk(1, i, t)
                           for i in range(len(ksl))],
                (0, 1): [lambda t: q_chunk(2, 0, t), lambda t: q_chunk(2, 1, t)]
                        + [lambda t, i=i: k_chunk(2, i, t)
                           for i in range(len(ksl))]
                        + [lambda t: q_chunk(0, 2, t), lambda t: q_chunk(0, 3, t)],
                (0, 2): [lambda t: q_chunk(1, 2, t), lambda t: q_chunk(1, 3, t),
                         lambda t: q_chunk(2, 2, t), lambda t: q_chunk(2, 3, t)],
                (1, 0): [lambda t, qc=qc: o_chunk(qc, t) for qc in range(0, 5)],
                (1, 1): [lambda t, qc=qc: o_chunk(qc, t) for qc in range(5, 8)],
                (1, 2): [],
            }

            # lead-in: just enough projection to unblock unit (0, 0)
            q_chunk(0, 0, tag="c")
            q_chunk(0, 1, tag="c")
            k_chunk(0, 0, tag="c")
            # only v0..v8 are needed before the C-phase; v0 borrows a ctx psum
            # slot so the first score tiles don't queue behind its eviction.
            v_chunk(0, tag="c")

            for qh, pair in units:
                q0 = qh * QH
                work = list(hidden[(qh, pair)])
                weights = [{}, {}]      # per-parity exp views for ctx

                # ---- S phase: row-tiled score singles + exp + hidden ----
                # per-(kc,qt) psum tile with E in cols 0:512, O in 512:1024:
                # ONE alloc-wait covers the E/O pair (so the row-tiled pair
                # issues back-to-back and runs concurrently in different PE
                # row-groups) and ONE exp op serves both parities, while
                # bufs=2 keeps the score->exp->score chain pipelined.
                for kc in range(KC):
                    for qt in range(2):
                        st = ps.tile([P, QH], F32, tag="s",
                                     name=f"st{qh}{pair}{kc}{qt}")
                        qsl = slice(q0 + qt * 512, q0 + (qt + 1) * 512)
                        nc.tensor.matmul(
                            st[:, 0:512],
                            kT_sb[0:64, pair, kc * P:(kc + 1) * P],
                            qT_sb[0:64, pair, qsl], start=True, stop=True)
                        nc.tensor.matmul(
                            st[:, 512:1024],
                            kT_sb[64:128, pair, kc * P:(kc + 1) * P],
                            qT_sb[64:128, pair, qsl], start=True, stop=True)
                        # exp: ACT or DVE (Schraudolph) by kv chunk
                        if kc in dve_kc:
                            sch_t = schp.tile([P, QH], F32, tag="sch",
                                              bufs=4,
                                              name=f"sch{qh}{pair}{kc}{qt}")
                            nc.vector.tensor_scalar(
                                sch_t[:], st[:],
                                SCH_SCALE, SCH_BIAS,
                                mybir.AluOpType.mult, mybir.AluOpType.add,
                            )
                            wv_ = sch_t.bitcast(MM_DT).rearrange(
                                "p (n two) -> p n two", two=2)[:, :, 0]
                        else:
                            # a unit's exp tiles stay live through its
                            # C-phase (in-order PE queue: an alloc waiting
                            # on a C-phase reader behind it would deadlock)
                            exp_t = expp.tile([P, QH], MM_DT, tag="exp",
                                              bufs=20,
                                              name=f"exp{qh}{pair}{kc}{qt}")
                            nc.scalar.activation(
                                exp_t[:], st[:],
                                mybir.ActivationFunctionType.Exp,
                                bias=pb_sb[:, kc, None], scale=0.125,
                            )
                            wv_ = exp_t
                        weights[0][(kc, qt)] = wv_[:, 0:512]
                        weights[1][(kc, qt)] = wv_[:, 512:1024]
                    # hidden projection / out-projection chunks; pop two
                    # per slot when the list would otherwise not fit. Pops
                    # start at kc==2 only: an o_chunk popped earlier would
                    # read ctx_sb rows the kc==2 pending_fin hasn't written.
                    if kc >= 2 and work:
                        work.pop(0)("c")
                        if len(work) > KC - 1 - kc:
                            work.pop(0)("c")
                while work:
                    work.pop(0)("c")

                # ---- C phase: col-tiled ctx accumulation (PURE: no other
                # matmuls may be emitted until the groups close) ----
                cE = ps.tile([P, QH], F32, tag="c", name=f"cE{qh}{pair}")
                cO = ps.tile([P, QH], F32, tag="c", name=f"cO{qh}{pair}")
                # M=128 single matmuls (col-tiled sums/dims pairs measured
                # SLOWER: same-row LDWEIGHTS can't overlap the in-flight
                # matmul, serializing at ~330ns vs 216ns streamed)
                for par, cX in ((0, cE), (1, cO)):
                    h = 2 * pair + par
                    for ckc in range(KC):
                        for qt in range(2):
                            sl = slice(qt * 512, (qt + 1) * 512)
                            nc.tensor.matmul(
                                cX[:, sl],
                                v_sb[:, ckc, 128 * h:128 * h + 128],
                                weights[par][(ckc, qt)],
                                start=(ckc == 0), stop=(ckc == KC - 1))

                unit_tail(pair, qh, cE, cO)

            # out-projection for the second q half (ACT's hwdge queue is
            # idle after the last exp; split the issue across both queues)
            for qc in range(8, 16):
                o_chunk(qc, dma=nc.scalar if qc % 2 else nc.sync)

    nc.compile()
    return nc


_cache: dict = {}

# test harnesses may set e.g. {"trace": True, "tmpdir": ...}; empty for grading
_run_opts: dict = {}
LAST_RES = None


def _get_nc(NKV: int):
    if NKV not in _cache:
        _cache[NKV] = _build(NKV)
    return _cache[NKV]


def kernel(query, key_, value, mask, q_w, q_b, k_w, k_b, v_w, v_b, o_w, o_b):
    query = np.asarray(query, np.float32)
    key_ = np.asarray(key_, np.float32)
    value = np.asarray(value, np.float32)
    mask = np.asarray(mask)
    q_w = np.asarray(q_w, np.float32)
    q_b = np.asarray(q_b, np.float32)
    k_w = np.asarray(k_w, np.float32)
    k_b = np.asarray(k_b, np.float32)
    v_w = np.asarray(v_w, np.float32)
    v_b = np.asarray(v_b, np.float32)
    o_w = np.asarray(o_w, np.float32)
    o_b = np.asarray(o_b, np.float32)

    counts = (mask != 0).sum(axis=1)
    NKV = max(P, int(-(-int(counts.max()) // P) * P))
    nc = _get_nc(NKV)

    in_maps = []
    for b in range(BS):
        idx = np.nonzero(mask[b])[0]
        cnt = len(idx)
        xk_g = np.zeros((NKV, DIM), np.float32)
        xv_g = np.zeros((NKV, DIM), np.float32)
        xk_g[:cnt] = key_[b][idx]
        xv_g[:cnt] = value[b][idx]
        xqT_b = np.ascontiguousarray(query[b].T).astype(MM_NP)
        xkT_b = np.ascontiguousarray(xk_g.T).astype(MM_NP)
        xvT_b = np.zeros((P * KIN_V, NKV), MM_NP)
        xvT_b[:DIM] = xv_g.T
        xvT_b[DIM] = 1.0
        pb_b = np.where(np.arange(NKV) < cnt, BIAS_CENTER, NEG).astype(np.float32)
        for g in range(2):
            sl = slice(DGRP * g, DGRP * (g + 1))
            in_maps.append({
                "xqT": xqT_b,
                "xkT": xkT_b,
                "xvT": xvT_b,
                "wqT": np.ascontiguousarray(q_w[sl].T).astype(MM_NP),
                "wkT": np.ascontiguousarray(k_w[sl].T).astype(MM_NP),
                "wvT": np.concatenate(
                    [v_w[sl].T, v_b[None, sl],
                     np.zeros((P - 1, DGRP), np.float32)], axis=0).astype(MM_NP),
                "woT": np.ascontiguousarray(o_w[:, sl].T).astype(MM_NP),
                "qb": q_b[sl].copy(),
                "kb": k_b[sl].copy(),
                "pb": pb_b,
            })

    res = run_bass_kernel_spmd(nc, in_maps, core_ids=list(range(N_CORES)),
                               **_run_opts)
    global LAST_RES
    LAST_RES = res
    out = np.empty((BS, SEQ, DIM), np.float32)
    for b in range(BS):
        out[b] = res.results[2 * b]["out"] + res.results[2 * b + 1]["out"] + o_b
    return out


# revision 18
# speedup vs baseline: 1.1820x; 1.0230x over previous
"""Multi-head self-attention (B=4, S=2048, D=768, H=12, dh=64) on 8 trn2 cores.

Sharding: core = b*2 + g  (b = batch 0..3, g = head-group of 6 heads).
Each core computes q/k/v projections for its 6 heads over the full sequence,
masked softmax attention, and a partial output projection (column slice of
o_w => row-parallel). Host sums the two partial outputs per batch element and
adds o_b once.

v3 design (vs the 220us baseline): PE work is cut ~30% via PE-array tiling,
with the attention unit split into phases so tiled matmuls never interleave
with open accumulation groups of a different tile mode (that pattern is
fatal on HW - NRT_EXEC_UNIT_UNRECOVERABLE):
  - S-phase: scores as K=64 row-tiled start/stop singles; the two heads of a
    pair (SBUF partitions 0..63 / 64..127) run in different PE row-groups
    CONCURRENTLY (~2x). kT is stored pair-stacked [128, pair, kv]; no
    zero-half padding, no kTz memset, single-op k eviction.
  - C-phase (pure, no hidden work inside): ctx as col-tiled open groups:
    sums matmul M=32 (v-lane col 0 = ones) -> psum rows 0..31, dims matmul
    M=64 (v-lane cols 64..127) -> psum rows 64..127; the two run in
    different PE col-groups concurrently (~2x). Score tiles stay [kv, q]
    so exp weights feed ctx as the moving operand with no transpose.
  - mask gather: only unmasked k positions (padded to a multiple of 128)
    are shipped/projected/exp'd; padding columns get a -1e30 per-partition
    bias inside the ACT exp (out = exp(scale*s + bias)).
  - exp: ACT for most kv chunks; DVE_KC chunks use a DVE Schraudolph
    tensor_scalar (bf16 bits in the low half of f32(k + 2^23), consumed as
    a stride-2 bitcast view) to keep ACT off the critical path.
  - out-projection chunks DMA psum -> DRAM directly (no DVE evict, no o_b
    broadcast; the host adds o_b once after summing the two partials).
  - hidden projection work (q/k/v proj, o proj) fills PE slack in S-phase
    slots and at phase boundaries only - every hidden chunk is a closed
    128-mode accumulation group, legal between tiled singles.
  - ONE psum pool for the whole program (tags "s" x2 + "c" x2 = 8 banks).
  - normalization tail as before: sums land in psum row 0 (readable by the
    custom-DVE reciprocal at partition offset 0), ctx dims in rows 64..127;
    the recip broadcast (K=128 ones-matmul) + normalize multiply are
    emitted in the NEXT unit's S-phase (128-mode, outside any C-phase).
"""

import numpy as np
import ml_dtypes

import concourse.bass as bass
import concourse.mybir as mybir
import concourse.tile as tile
from concourse import bacc
from concourse.bass_utils import run_bass_kernel_spmd

BS, SEQ, DIM, NH = 4, 2048, 768, 12
DH = 64
HEADS = 6            # heads per core
NPAIR = 3            # head pairs per core
DGRP = HEADS * DH    # 384
N_CORES = 8
P = 128
QH = 1024            # q-half width in the attention loop
KIN = DIM // P       # 6 contraction chunks for q/k proj
KIN_V = 6            # 768 v-proj contraction inputs (bias via DVE evict)

F32 = mybir.dt.float32
BF16 = mybir.dt.bfloat16

MM_DT = BF16
MM_NP = ml_dtypes.bfloat16

NEG = -1.0e30
# ACT-path exp bias: centers the exact exp against the DVE piecewise-linear
# exp2 (max log2 ratio 0.08607 -> shift both means by half of that).
BIAS_CENTER = 0.0430365 * 0.6931471805599453   # = 0.0298296 (natural log)
# DVE Schraudolph constants: bf16 bits k = s * (128*log2e/8) + 16256,
# computed as  f32( s*23.08312 + (16256 + 2^23) )  -- the 2^23 add rounds
# k to the nearest integer in the f32 mantissa; bf16 bits are the LOW 16
# bits of the resulting f32 (element 0 of the bitcast pair).
SCH_SCALE = 128.0 * 1.4426950408889634 / 8.0   # 23.083120654
SCH_BIAS = 16256.0 + 8388608.0

# kv chunks whose exp runs on DVE instead of ACT (never the last, padded one)
DVE_KC = (3, 6)


def _build(NKV: int):
    """Build the per-core Bass program, parameterized by padded kv length."""
    KC = NKV // P          # kv chunks
    dve_kc = set(c for c in DVE_KC if c < KC - 1)

    nc = bacc.Bacc(None, target_bir_lowering=False, debug=False)

    xqT = nc.declare_dram_parameter("xqT", [DIM, SEQ], MM_DT, isOutput=False)
    xkT = nc.declare_dram_parameter("xkT", [DIM, NKV], MM_DT, isOutput=False)
    xvT = nc.declare_dram_parameter("xvT", [P * KIN_V, NKV], MM_DT, isOutput=False)
    wqT = nc.declare_dram_parameter("wqT", [DIM, DGRP], MM_DT, isOutput=False)
    wkT = nc.declare_dram_parameter("wkT", [DIM, DGRP], MM_DT, isOutput=False)
    wvT = nc.declare_dram_parameter("wvT", [P * KIN_V, DGRP], MM_DT, isOutput=False)
    woT = nc.declare_dram_parameter("woT", [DGRP, DIM], MM_DT, isOutput=False)
    qb = nc.declare_dram_parameter("qb", [DGRP], F32, isOutput=False)
    vb = nc.declare_dram_parameter("vb", [DGRP], F32, isOutput=False)
    kb = nc.declare_dram_parameter("kb", [DGRP], F32, isOutput=False)
    pb = nc.declare_dram_parameter("pb", [NKV], F32, isOutput=False)
    out = nc.declare_dram_parameter("out", [SEQ, DIM], F32, isOutput=True)

    xqT_r = xqT.rearrange("(kk pi) n -> pi kk n", pi=P)
    xkT_r = xkT.rearrange("(kk pi) n -> pi kk n", pi=P)
    xvT_r = xvT.rearrange("(kk pi) n -> pi kk n", pi=P)
    wqT_r = wqT.rearrange("(kk pi) n -> pi kk n", pi=P)
    wkT_r = wkT.rearrange("(kk pi) n -> pi kk n", pi=P)
    wvT_r = wvT.rearrange("(kk pi) n -> pi kk n", pi=P)
    woT_r = woT.rearrange("(kk pi) n -> pi kk n", pi=P)
    qb_r = qb.rearrange("(m pi) -> pi m", pi=P)
    kb_r = kb.rearrange("(m pi) -> pi m", pi=P)
    pb_r = pb.rearrange("(c pi) -> pi c", pi=P)

    # k-proj slices along kv
    ksl = []
    o = 0
    while o < NKV:
        w = min(512, NKV - o)
        ksl.append((o, w))
        o += w

    with tile.TileContext(nc) as tc:
        with (
            tc.tile_pool(name="const", bufs=1) as const,
            tc.tile_pool(name="persist", bufs=1) as persist,
            tc.tile_pool(name="expp", bufs=4) as expp,
            tc.tile_pool(name="schp", bufs=4) as schp,
            tc.tile_pool(name="outp", bufs=5) as outp,
            tc.tile_pool(name="ps", bufs=2, space="PSUM") as ps,
        ):
            # ---- DMA order = first-use order; wq/xq0 split so the first
            # q-proj matmuls start as early as possible ----
            # m=0 (pair 0) slices first: the lead-in q/k chunks for unit
            # (0,0) gate the whole pipeline, so ship exactly what they read
            wq_sb = const.tile([P, KIN, DGRP], MM_DT)
            nc.sync.dma_start(wq_sb[:, 0:3, 0:P], wqT_r[:, 0:3, 0:P])
            xq_sb = persist.tile([P, KIN, SEQ], MM_DT)
            nc.sync.dma_start(xq_sb[:, 0:3, 0:512], xqT_r[:, 0:3, 0:512])
            nc.scalar.dma_start(wq_sb[:, 3:6, 0:P], wqT_r[:, 3:6, 0:P])
            nc.scalar.dma_start(xq_sb[:, 3:6, 0:512], xqT_r[:, 3:6, 0:512])
            qb_sb = const.tile([P, 3], F32)
            nc.sync.dma_start(qb_sb[:], qb_r)
            nc.scalar.dma_start(xq_sb[:, 0:3, 512:1024], xqT_r[:, 0:3, 512:1024])
            nc.sync.dma_start(xq_sb[:, 3:6, 512:1024], xqT_r[:, 3:6, 512:1024])
            wk_sb = const.tile([P, KIN, DGRP], MM_DT)
            nc.scalar.dma_start(wk_sb[:, 0:3, 0:P], wkT_r[:, 0:3, 0:P])
            nc.sync.dma_start(wk_sb[:, 3:6, 0:P], wkT_r[:, 3:6, 0:P])
            xk_sb = persist.tile([P, KIN, NKV], MM_DT)
            o0, w = ksl[0]
            nc.scalar.dma_start(xk_sb[:, 0:3, o0:o0 + w], xkT_r[:, 0:3, o0:o0 + w])
            nc.sync.dma_start(xk_sb[:, 3:6, o0:o0 + w], xkT_r[:, 3:6, o0:o0 + w])
            kb_sb = const.tile([P, 3], F32)
            nc.sync.dma_start(kb_sb[:], kb_r)
            # remaining q/k weight columns (pairs 1 and 2)
            nc.scalar.dma_start(wq_sb[:, 0:3, P:DGRP], wqT_r[:, 0:3, P:DGRP])
            nc.sync.dma_start(wq_sb[:, 3:6, P:DGRP], wqT_r[:, 3:6, P:DGRP])
            nc.scalar.dma_start(wk_sb[:, 0:3, P:DGRP], wkT_r[:, 0:3, P:DGRP])
            nc.sync.dma_start(wk_sb[:, 3:6, P:DGRP], wkT_r[:, 3:6, P:DGRP])
            pb_sb = const.tile([P, KC], F32)
            nc.sync.dma_start(pb_sb[:], pb_r)
            wv_sb = const.tile([P, KIN_V, DGRP], MM_DT)
            nc.sync.dma_start(wv_sb[:], wvT_r)
            vb_row = const.tile([1, DGRP], F32)
            nc.sync.dma_start(vb_row[:], vb[None, :])
            vb_bc = const.tile([P, DGRP], F32)
            nc.gpsimd.partition_broadcast(vb_bc[:], vb_row[:])
            xv_sb = persist.tile([P, KIN_V, NKV], MM_DT)
            nc.sync.dma_start(xv_sb[:, :, 0:256], xvT_r[:, :, 0:256])
            if len(ksl) > 1:
                o0, w = ksl[1]
                nc.sync.dma_start(xk_sb[:, :, o0:o0 + w], xkT_r[:, :, o0:o0 + w])
            nc.sync.dma_start(xv_sb[:, :, 256:640], xvT_r[:, :, 256:640])
            for o0, w in ksl[2:]:
                nc.sync.dma_start(xk_sb[:, :, o0:o0 + w], xkT_r[:, :, o0:o0 + w])
            nc.sync.dma_start(xv_sb[:, :, 640:NKV], xvT_r[:, :, 640:NKV])
            wo_sb = const.tile([P, 3, DIM], MM_DT)
            nc.sync.dma_start(wo_sb[:], woT_r)
            nc.sync.dma_start(xq_sb[:, :, 1024:1536], xqT_r[:, :, 1024:1536])
            nc.sync.dma_start(xq_sb[:, :, 1536:2048], xqT_r[:, :, 1536:2048])

            # dummy exp: pulls the ~2.7us ACT_TABLE_LOAD into the DMA ramp
            warm_t = const.tile([P, 1], F32)
            nc.scalar.activation(warm_t[:], qb_sb[:, 0:1],
                                 mybir.ActivationFunctionType.Exp)

            # ---- persistent activations ----
            qT_sb = persist.tile([P, NPAIR, SEQ], MM_DT)
            # kT pair-stacked: rows 0..63 = head 2m, rows 64..127 = head 2m+1
            kT_sb = persist.tile([P, NPAIR, NKV], MM_DT)
            # v lanes are 128 wide per head: col 0 = ones (sums matmul lhsT is
            # cols 0..31 -> softmax sums land in psum row 0, readable by the
            # custom-DVE recip), cols 64..127 = the 64 v dims (dims matmul
            # lhsT -> psum rows 64..127). cols 1..63 never read.
            v_sb = persist.tile([P, KC, HEADS * 128], MM_DT)
            ctx_sb = persist.tile([P, NPAIR, SEQ], MM_DT)
            # recip tiles (row 0 = 1/sums, rows 1..127 = 0 so the K=128
            # ones-matmul broadcast never multiplies 0 by inf/nan garbage)
            recipA = persist.tile([P, QH], F32)
            recipB = persist.tile([P, QH], F32)
            # partition-broadcast targets (Pool writes, DVE mult reads)
            rbA_sb = persist.tile([P, QH], F32)
            rbB_sb = persist.tile([P, QH], F32)
            for h in range(HEADS):
                nc.vector.memset(v_sb[:, :, 128 * h], 1.0)

            # ---- emission helpers (all share the "s"/"c" psum slots) ----
            def q_chunk(m, nt, tag="c"):
                ps_t = ps.tile([P, QH], F32, tag=tag, name=f"q{m}{nt}")
                for kk in range(KIN):
                    nc.tensor.matmul(
                        ps_t[:, 0:512],
                        wq_sb[:, kk, m * P:(m + 1) * P],
                        xq_sb[:, kk, nt * 512:(nt + 1) * 512],
                        start=(kk == 0), stop=(kk == KIN - 1),
                    )
                nc.vector.tensor_scalar_add(
                    qT_sb[:, m, nt * 512:(nt + 1) * 512], ps_t[:, 0:512],
                    qb_sb[:, m, None],
                )

            def k_chunk(m, isl, tag="c"):
                o0, w = ksl[isl]
                ps_t = ps.tile([P, QH], F32, tag=tag, name=f"k{m}{isl}")
                for kk in range(KIN):
                    nc.tensor.matmul(
                        ps_t[:, 0:w],
                        wk_sb[:, kk, m * P:(m + 1) * P],
                        xk_sb[:, kk, o0:o0 + w],
                        start=(kk == 0), stop=(kk == KIN - 1),
                    )
                nc.vector.tensor_scalar_add(
                    kT_sb[:, m, o0:o0 + w], ps_t[:, 0:w],
                    kb_sb[:, m, None],
                )

            def v_chunk(c, tag="c"):
                ps_t = ps.tile([P, QH], F32, tag=tag, name=f"v{c}")
                for kk in range(KIN_V):
                    nc.tensor.matmul(
                        ps_t[:, 0:DGRP],
                        xv_sb[:, kk, c * P:(c + 1) * P],
                        wv_sb[:, kk, :],
                        start=(kk == 0), stop=(kk == KIN_V - 1),
                    )
                # strided evict into slots 64..127 of the 128-wide head
                # lanes, adding the per-dim v bias (broadcast tile) in-op
                nc.vector.tensor_tensor(
                    v_sb[:, c, :].rearrange("p (h x) -> p h x", x=128)[:, :, 64:128],
                    ps_t[:, 0:DGRP].rearrange("p (h x) -> p h x", x=64),
                    vb_bc[:].rearrange("p (h x) -> p h x", x=64),
                    mybir.AluOpType.add,
                )

            def o_chunk(qc, tag="c", dma=None):
                ps_t = ps.tile([P, QH], F32, tag=tag, name=f"o{qc}")
                for kk in range(3):
                    for n0, nsz in ((0, 512), (512, 256)):
                        nc.tensor.matmul(
                            ps_t[:, n0:n0 + nsz],
                            ctx_sb[:, kk, qc * P:(qc + 1) * P],
                            wo_sb[:, kk, n0:n0 + nsz],
                            start=(kk == 0), stop=(kk == 2),
                        )
                # DMA cannot read PSUM: stage through SBUF with a plain copy
                # (host adds o_b once after summing the two partials)
                o_t = outp.tile([P, DIM], F32, tag="o", name=f"ot{qc}")
                nc.vector.tensor_copy(out=o_t[:], in_=ps_t[:, 0:DIM])
                (dma or nc.sync).dma_start(out[qc * P:(qc + 1) * P, :], o_t[:])

            def unit_tail(pair, qh, cE, cO):
                # normalization tail: approx recip off psum row 0 (custom-DVE
                # ops require partition-0 inputs), partition-broadcast of the
                # recip row on the IDLE Pool engine (no PE ones-matmul), ctx
                # dims staged to SBUF (TensorTensor allows only one PSUM
                # input), one multiply per head into ctx_sb.
                q0 = qh * QH
                nc.vector.reciprocal_approx_fast(
                    out=recipA[0:1, :], in_=cE[0:1, :])
                nc.gpsimd.partition_broadcast(rbA_sb[:], recipA[0:1, :])
                nc.vector.reciprocal_approx_fast(
                    out=recipB[0:1, :], in_=cO[0:1, :])
                nc.gpsimd.partition_broadcast(rbB_sb[:], recipB[0:1, :])
                nc.vector.tensor_tensor(
                    ctx_sb[0:DH, pair, q0:q0 + QH],
                    cE[64:128, :], rbA_sb[0:DH, :], mybir.AluOpType.mult,
                )
                nc.vector.tensor_tensor(
                    ctx_sb[DH:P, pair, q0:q0 + QH],
                    cO[64:128, :], rbB_sb[DH:P, :], mybir.AluOpType.mult,
                )

            units = [(qh, pr) for qh in range(2) for pr in range(NPAIR)]
            hidden: dict = {
                # order matters: v chunk c must be emitted before the C-phase
                # that reads it (same unit); k/q chunks for a unit must be
                # emitted in an EARLIER unit (in-order PE queue: a score
                # matmul cannot wait on projection work emitted behind it).
                (0, 0): [lambda t: v_chunk(1, t), lambda t: v_chunk(2, t),
                         lambda t: k_chunk(0, 1, t), lambda t: v_chunk(3, t),
                         (lambda t: k_chunk(0, 2, t)) if len(ksl) > 2
                         else (lambda t: v_chunk(4, t))]
                        + [lambda t, c=c: v_chunk(c, t)
                           for c in range(4 if len(ksl) > 2 else 5, KC)]
                        + [lambda t: q_chunk(1, 0, t), lambda t: q_chunk(1, 1, t)]
                        + [lambda t, i=i: k_chunk(1, i, t)
                           for i in range(len(ksl))],
                (0, 1): [lambda t: q_chunk(2, 0, t), lambda t: q_chunk(2, 1, t)]
                        + [lambda t, i=i: k_chunk(2, i, t)
                           for i in range(len(ksl))]
                        + [lambda t: q_chunk(0, 2, t), lambda t: q_chunk(0, 3, t)],
                (0, 2): [lambda t: q_chunk(1, 2, t), lambda t: q_chunk(1, 3, t),
                         lambda t: q_chunk(2, 2, t), lambda t: q_chunk(2, 3, t)],
                (1, 0): [lambda t, qc=qc: o_chunk(qc, t) for qc in range(0, 5)],
                (1, 1): [lambda t, qc=qc: o_chunk(qc, t) for qc in range(5, 8)],
                (1, 2): [],
            }

            # lead-in: just enough projection to unblock unit (0, 0)
            q_chunk(0, 0, tag="c")
            q_chunk(0, 1, tag="c")
            k_chunk(0, 0, tag="c")
            # only v0..v8 are needed before the C-phase; v0 borrows a ctx psum
            # slot so the first score tiles don't queue behind its eviction.
            v_chunk(0, tag="c")

            for qh, pair in units:
                q0 = qh * QH
                work = list(hidden[(qh, pair)])
                weights = [{}, {}]      # per-parity exp views for ctx

                # ---- S phase: row-tiled score singles + exp + hidden ----
                # per-(kc,qt) psum tile with E in cols 0:512, O in 512:1024:
                # ONE alloc-wait covers the E/O pair (so the row-tiled pair
                # issues back-to-back and runs concurrently in different PE
                # row-groups) and ONE exp op serves both parities, while
                # bufs=2 keeps the score->exp->score chain pipelined.
                for kc in range(KC):
                    for qt in range(2):
                        st = ps.tile([P, QH], F32, tag="s",
                                     name=f"st{qh}{pair}{kc}{qt}")
                        qsl = slice(q0 + qt * 512, q0 + (qt + 1) * 512)
                        nc.tensor.matmul(
                            st[:, 0:512],
                            kT_sb[0:64, pair, kc * P:(kc + 1) * P],
                            qT_sb[0:64, pair, qsl], start=True, stop=True)
                        nc.tensor.matmul(
                            st[:, 512:1024],
                            kT_sb[64:128, pair, kc * P:(kc + 1) * P],
                            qT_sb[64:128, pair, qsl], start=True, stop=True)
                        # exp: ACT or DVE (Schraudolph) by kv chunk
                        if kc in dve_kc:
                            sch_t = schp.tile([P, QH], F32, tag="sch",
                                              bufs=6,
                                              name=f"sch{qh}{pair}{kc}{qt}")
                            nc.vector.tensor_scalar(
                                sch_t[:], st[:],
                                SCH_SCALE, SCH_BIAS,
                                mybir.AluOpType.mult, mybir.AluOpType.add,
                            )
                            wv_ = sch_t.bitcast(MM_DT).rearrange(
                                "p (n two) -> p n two", two=2)[:, :, 0]
                        else:
                            # a unit's exp tiles stay live through its
                            # C-phase (in-order PE queue: an alloc waiting
                            # on a C-phase reader behind it would deadlock)
                            exp_t = expp.tile([P, QH], MM_DT, tag="exp",
                                              bufs=17,
                                              name=f"exp{qh}{pair}{kc}{qt}")
                            nc.scalar.activation(
                                exp_t[:], st[:],
                                mybir.ActivationFunctionType.Exp,
                                bias=pb_sb[:, kc, None], scale=0.125,
                            )
                            wv_ = exp_t
                        weights[0][(kc, qt)] = wv_[:, 0:512]
                        weights[1][(kc, qt)] = wv_[:, 512:1024]
                    # hidden projection / out-projection chunks; pop two
                    # per slot when the list would otherwise not fit. Pops
                    # start at kc==2 only: an o_chunk popped earlier would
                    # read ctx_sb rows the kc==2 pending_fin hasn't written.
                    if kc >= 2 and work:
                        work.pop(0)("c")
                        if len(work) > KC - 1 - kc:
                            work.pop(0)("c")
                while work:
                    work.pop(0)("c")

                # ---- C phase: col-tiled ctx accumulation (PURE: no other
                # matmuls may be emitted until the groups close) ----
                cE = ps.tile([P, QH], F32, tag="c", name=f"cE{qh}{pair}")
                cO = ps.tile([P, QH], F32, tag="c", name=f"cO{qh}{pair}")
                # M=128 single matmuls (col-tiled sums/dims pairs measured
                # SLOWER: same-row LDWEIGHTS can't overlap the in-flight
                # matmul, serializing at ~330ns vs 216ns streamed)
                for par, cX in ((0, cE), (1, cO)):
                    h = 2 * pair + par
                    for ckc in range(KC):
                        for qt in range(2):
                            sl = slice(qt * 512, (qt + 1) * 512)
                            nc.tensor.matmul(
                                cX[:, sl],
                                v_sb[:, ckc, 128 * h:128 * h + 128],
                                weights[par][(ckc, qt)],
                                start=(ckc == 0), stop=(ckc == KC - 1))

                unit_tail(pair, qh, cE, cO)

            # out-projection for the second q half (ACT's hwdge queue is
            # idle after the last exp; split the issue across both queues)
            for qc in range(8, 16):
                o_chunk(qc, dma=nc.scalar if qc % 2 else nc.sync)

    nc.compile()
    return nc


_cache: dict = {}

# test harnesses may set e.g. {"trace": True, "tmpdir": ...}; empty for grading
_run_opts: dict = {}
LAST_RES = None


def _get_nc(NKV: int):
    if NKV not in _cache:
        _cache[NKV] = _build(NKV)
    return _cache[NKV]


def kernel(query, key_, value, mask, q_w, q_b, k_w, k_b, v_w, v_b, o_w, o_b):
    query = np.asarray(query, np.float32)
    key_ = np.asarray(key_, np.float32)
    value = np.asarray(value, np.float32)
    mask = np.asarray(mask)
    q_w = np.asarray(q_w, np.float32)
    q_b = np.asarray(q_b, np.float32)
    k_w = np.asarray(k_w, np.float32)
    k_b = np.asarray(k_b, np.float32)
    v_w = np.asarray(v_w, np.float32)
    v_b = np.asarray(v_b, np.float32)
    o_w = np.asarray(o_w, np.float32)
    o_b = np.asarray(o_b, np.float32)

    counts = (mask != 0).sum(axis=1)
    NKV = max(P, int(-(-int(counts.max()) // P) * P))
    nc = _get_nc(NKV)

    in_maps = []
    for b in range(BS):
        idx = np.nonzero(mask[b])[0]
        cnt = len(idx)
        xk_g = np.zeros((NKV, DIM), np.float32)
        xv_g = np.zeros((NKV, DIM), np.float32)
        xk_g[:cnt] = key_[b][idx]
        xv_g[:cnt] = value[b][idx]
        xqT_b = np.ascontiguousarray(query[b].T).astype(MM_NP)
        xkT_b = np.ascontiguousarray(xk_g.T).astype(MM_NP)
        xvT_b = np.ascontiguousarray(xv_g.T).astype(MM_NP)
        pb_b = np.where(np.arange(NKV) < cnt, BIAS_CENTER, NEG).astype(np.float32)
        for g in range(2):
            sl = slice(DGRP * g, DGRP * (g + 1))
            in_maps.append({
                "xqT": xqT_b,
                "xkT": xkT_b,
                "xvT": xvT_b,
                "wqT": np.ascontiguousarray(q_w[sl].T).astype(MM_NP),
                "wkT": np.ascontiguousarray(k_w[sl].T).astype(MM_NP),
                "wvT": np.ascontiguousarray(v_w[sl].T).astype(MM_NP),
                "woT": np.ascontiguousarray(o_w[:, sl].T).astype(MM_NP),
                "qb": q_b[sl].copy(),
                "vb": v_b[sl].copy(),
                "kb": k_b[sl].copy(),
                "pb": pb_b,
            })

    res = run_bass_kernel_spmd(nc, in_maps, core_ids=list(range(N_CORES)),
                               **_run_opts)
    global LAST_RES
    LAST_RES = res
    out = np.empty((BS, SEQ, DIM), np.float32)
    for b in range(BS):
        out[b] = res.results[2 * b]["out"] + res.results[2 * b + 1]["out"] + o_b
    return out
